# revision 44
# baseline (speedup 1.0000x reference)
"""ArcFace multi-head-sharded loss on 8 TRN2 NeuronCores.

Strategy: shard the (64, 2048, 256) weight table over the group axis —
each core owns 8 groups. Samples are routed host-side to the core owning
their group (host routing replaces the all-to-all). Weight rows are
l2-normalized host-side and quantized to fp8e4 (x16 pre-scale to stay in
the normal range), so the device only does:

  - stream its 8 weight groups (4MB fp8) from HBM; DMA triggers alternate
    between the two HW-DGE queues (sync + scalar) so descriptor
    generation is not serialized on one engine,
  - mains: cos_raw(b, c) = <xq_b, wq_c> on PE (fp8 x fp8 -> f32 PSUM),
  - exp with fused accumulation over the class axis (scale folds the
    1/256 quantization scale and the ArcFace scale 64),
  - target logit via a per-row dot with the host-gathered target weight
    row (xw . wtar, 256-wide DVE reduce),
  - the margin + CE epilogue on [128,T] vectors (both tiles batched),
  - one partial-loss scalar out (sum of -logp/B over its samples).

Host: sums the 8 scalars. ~4MB HBM traffic per core => memory-bound.

Samples are packed into "bands" of NG=32 partition rows, one band per
weight group (plus overflow bands), 4 bands per 128-row sample tile.
"""

import sys
import numpy as np
import ml_dtypes

BF16 = ml_dtypes.bfloat16
FP8 = ml_dtypes.float8_e4m3

_TRN_REPO = "/opt/trn_rl_repo"
if _TRN_REPO not in sys.path:
    sys.path.insert(0, _TRN_REPO)

# problem config (hardcoded per spec)
B, E, G, C = 512, 256, 64, 2048
NCORES = 8
GPC = G // NCORES        # weight groups per core
NG = 32                  # sample slots per band
BPT = 128 // NG          # bands per 128-partition sample tile
KE = E // 128            # contraction chunks
NCC = C // 512           # 512-col chunks per group
SCALE = 64.0
MARGIN = 0.5
COS_M = float(np.cos(MARGIN))
SIN_M = float(np.sin(MARGIN))
THETA = float(np.cos(np.pi - MARGIN))
SINMM = float(np.sin(np.pi - MARGIN) * MARGIN)
EPS = 1e-12
WS = 16.0                # fp8 weight pre-scale; PSUM = WS * cos (x is bf16)
NAUX = 2 * E + KE * 128  # per-tile aux row: xw | wtar | x-transposed
LB_SHIFT = float(40.0 * np.log(2.0))  # ln-range shift, re-added host-side
DOUBLE_ROW = True       # fp8 DoubleRow perf mode for the mains

_graph_cache = {}


def _build(nb, double_row=DOUBLE_ROW):
    """Build the per-core Bass graph for nb weight bands (nb % BPT == 0)."""
    from contextlib import ExitStack
    import concourse.bacc as bacc
    import concourse.tile as tile
    from concourse import mybir

    f32 = mybir.dt.float32
    bf16 = mybir.dt.bfloat16
    fp8 = mybir.dt.float8e4
    i32 = mybir.dt.int32
    A = mybir.AluOpType
    AF = mybir.ActivationFunctionType

    T = nb // BPT
    nc = bacc.Bacc(None)

    # band-pair transfers (8KB DMA lines) except the last two bands, which
    # are split into per-chunk pieces so the tail's matmuls unlock
    # incrementally as the stream finishes instead of all at once
    pairs = [(b, b + 1) for b in range(0, nb - 2, 2)]
    piece_bands = [nb - 2, nb - 1]
    wt_exts = [
        nc.declare_dram_parameter(f"wt{gi}", [128, 2, KE, C], fp8,
                                  isOutput=False)
        for gi in range(len(pairs))
    ]
    wtp_exts = {
        (b, cc): nc.declare_dram_parameter(f"wtp{b}_{cc}", [128, KE, 512], fp8,
                                           isOutput=False)
        for b in piece_bands for cc in range(NCC)
    }
    aux_ext = nc.declare_dram_parameter("aux", [128, T, NAUX], bf16, isOutput=False)
    out_ext = nc.declare_dram_parameter("out", [128, T], f32, isOutput=True)

    with tile.TileContext(nc) as tc, ExitStack() as ctx:
        wpool = ctx.enter_context(tc.tile_pool(name="w", bufs=len(pairs)))
        ppool = ctx.enter_context(tc.tile_pool(name="wp", bufs=2 * NCC))
        cpool = ctx.enter_context(tc.tile_pool(name="const", bufs=1))
        vpool = ctx.enter_context(tc.tile_pool(name="vec", bufs=2))
        pmain = ctx.enter_context(tc.tile_pool(name="pmain", bufs=8, space="PSUM"))

        g_tiles = []
        for gi, g in enumerate(pairs):
            gt_ = wpool.tile([128, 2, KE, C], fp8, tag="wt", name=f"wt{gi}")
            g_tiles.append(gt_)
        p_tiles = {}
        for b in piece_bands:
            for cc in range(NCC):
                p_tiles[(b, cc)] = ppool.tile([128, KE, 512], fp8, tag="wp",
                                              name=f"wp{b}_{cc}")

        def rhs_ap(b, k, cc):
            if b in piece_bands:
                return p_tiles[(b, cc)][:, k, :]
            return g_tiles[b // 2][:, b % 2, k, 512 * cc: 512 * cc + 512]

        aux_sb = cpool.tile([128, T, NAUX], bf16, tag="aux")

        def xt_ap(t, k, j):
            # transposed bf16 x inside aux: [p, t, 2E + k*128 + r]
            off = 2 * E + 128 * k
            return aux_sb[:, t, off + NG * j: off + NG * (j + 1)]

        # preload the natural_log_exp_and_others ACT table set (exp, ln)
        nc.scalar.add_instruction(mybir.InstLoadActFuncSet(
            name="preload-actset-6", act_func_set_id=6, ins=[], outs=[]))

        # Everything rides the sync ring: its queue has nothing else, so
        # ring-depth stalls on later triggers block nothing, and the small
        # aux input isn't starved behind the band stream by engine
        # arbitration. The first band pair leads so the weight stream starts
        # immediately; aux (margin + PE x-operand) follows it. The last two
        # bands stream as 8 chunk pieces, second-to-last band first.
        nc.sync.dma_start(out=g_tiles[0][:], in_=wt_exts[0][:])
        nc.sync.dma_start(out=aux_sb[:], in_=aux_ext[:])
        for gi in range(1, len(pairs)):
            nc.sync.dma_start(out=g_tiles[gi][:], in_=wt_exts[gi][:])
        for b in piece_bands:
            for cc in range(NCC):
                nc.sync.dma_start(out=p_tiles[(b, cc)][:], in_=wtp_exts[(b, cc)][:])

        # margin pre-compute, both tiles batched as [128, T] columns:
        # t = <xn, wn_target>; ft = t>theta ? t*cos_m - sqrt(1-t^2)*sin_m
        #                                  : t - sinmm   (labels always valid)
        tcos = cpool.tile([128, T], f32, tag="tcos")
        for t in range(T):
            tscr = vpool.tile([128, E], f32, tag="tscr")
            nc.vector.tensor_tensor(tscr[:], aux_sb[:, t, 0:E],
                                    aux_sb[:, t, E:2 * E], A.mult)
            nc.vector.reduce_sum(tcos[:, t:t + 1], tscr[:], axis=mybir.AxisListType.X)
        t2 = vpool.tile([128, T], f32, tag="t2")
        nc.vector.tensor_tensor(t2[:], tcos[:], tcos[:], A.mult)
        nc.vector.tensor_scalar(t2[:], t2[:], -1.0, 1.0, op0=A.mult, op1=A.add)
        nc.vector.tensor_scalar_max(t2[:], t2[:], 0.0)
        # sin_t = z*rsqrt(z): Quake seed + 2 Newton iterations on DVE
        yrs = vpool.tile([128, T], f32, tag="yrs")
        yi = yrs.bitcast(i32)
        nc.vector.tensor_scalar(yi[:], t2.bitcast(i32)[:], 1, None, op0=A.arith_shift_right)
        nc.vector.tensor_scalar(yi[:], yi[:], -1, 0x5F3759DF, op0=A.mult, op1=A.add)
        hz = vpool.tile([128, T], f32, tag="hz")
        nc.vector.tensor_scalar_mul(hz[:], t2[:], 0.5)
        y2 = vpool.tile([128, T], f32, tag="y2")
        for _ in range(2):
            nc.vector.tensor_tensor(y2[:], yrs[:], yrs[:], A.mult)
            nc.vector.tensor_tensor(y2[:], y2[:], hz[:], A.mult)
            nc.vector.tensor_scalar(y2[:], y2[:], -1.0, 1.5, op0=A.mult, op1=A.add)
            nc.vector.tensor_tensor(yrs[:], yrs[:], y2[:], A.mult)
        sint = vpool.tile([128, T], f32, tag="sint")
        nc.vector.tensor_tensor(sint[:], t2[:], yrs[:], A.mult)
        ctm = vpool.tile([128, T], f32, tag="ctm")
        nc.vector.tensor_scalar_mul(ctm[:], tcos[:], COS_M)
        sinm = vpool.tile([128, T], f32, tag="sinm")
        nc.vector.tensor_scalar_mul(sinm[:], sint[:], SIN_M)
        nc.vector.tensor_tensor(ctm[:], ctm[:], sinm[:], A.subtract)
        tms = vpool.tile([128, T], f32, tag="tms")
        nc.vector.tensor_scalar_add(tms[:], tcos[:], -SINMM)
        gt = vpool.tile([128, T], i32, tag="gt")
        nc.vector.tensor_scalar(gt[:], tcos[:], THETA, None, op0=A.is_gt)
        ft = vpool.tile([128, T], f32, tag="ft")
        nc.vector.select(ft[:], gt[:], ctm[:], tms[:])
        # the sumexp path exponentiates bf16-rounded WS*cos values, so the
        # subtracted target term must go through the same rounding
        tbq = vpool.tile([128, T], bf16, tag="tbq")
        nc.vector.tensor_scalar_mul(tbq[:], tcos[:], WS)
        tf64 = cpool.tile([128, 3 * T], f32, tag="tf64")
        nc.vector.tensor_scalar_mul(tf64[:, 0:T], tbq[:], SCALE / WS)
        nc.vector.tensor_scalar_mul(tf64[:, T:2 * T], ft[:], SCALE)
        # -64ft - LB_SHIFT: the shift keeps ln's argument inside the scalar
        # engine's +-2^64 range; the host adds LB_SHIFT back per sample
        nc.vector.tensor_scalar(tf64[:, 2 * T:3 * T], ft[:], -SCALE, -LB_SHIFT,
                                op0=A.mult, op1=A.add)
        # eb = exp(tf64) is emitted lazily in the first tail so it sits on
        # the ACT queue AFTER tile0's chunk exps (it's only needed at fold
        # time, and emitting it early would chain the exps behind the margin
        # pre-compute).
        # per tile t: exp(64t)=eb[:,t], exp(64ft)=eb[:,T+t], exp(-64ft)=eb[:,2T+t]
        # ebd4 = (exp(64ft) - exp(64t)) / NCC folds the target-logit swap into
        # the per-chunk-sum reduction bias
        eb = cpool.tile([128, 3 * T], f32, tag="eb")
        eb2d = cpool.tile([128, T], f32, tag="eb2d")
        eb_emitted = []

        def emit_eb():
            if eb_emitted:
                return
            eb_emitted.append(True)
            nc.scalar.activation(eb[:], tf64[:], AF.Exp)
            # eb2d = exp(-64ft-SHIFT) * (exp(64ft) - exp(64t)): the ln bias
            # that swaps the target logit inside the scaled softmax sum
            nc.vector.tensor_tensor(eb2d[:], eb[:, T:2 * T], eb[:, 0:T], A.subtract)
            nc.vector.tensor_tensor(eb2d[:], eb2d[:], eb[:, 2 * T:3 * T], A.mult)

        # exp(escale * psum) = exp(64*cos); psum = WS*cos. Passed as a
        # per-partition AP (matches the fast ACT path; an immediate scale
        # measured ~2x slower per column).
        escale = cpool.tile([128, 1], f32, tag="escale")
        nc.vector.memset(escale[:], SCALE / WS)

        cps_t = {}
        ses_t = {}
        cosbf_t = {}

        def mm(t, cc, k, j, cps):
            nc.tensor.matmul(
                cps[cc][NG * j:NG * (j + 1), :],
                xt_ap(t, k, j),
                rhs_ap(BPT * t + j, k, cc),
                start=(k == 0), stop=(k == KE - 1),
                tile_position=(0, NG * j),
            )

        def emit_mains(t, defer_last):
            """Matmul order (cc, k, j): j innermost so the 4 bands' matmuls
            run concurrently on distinct PE column quadrants, cc outermost so
            PSUM chunks complete (and exp) one at a time. For the last tile
            the final band (latest DMA) is deferred per-chunk so the in-order
            PE queue drains all other work before waiting on it, and each
            chunk still completes (and exps) as early as possible."""
            cps_t[t] = [pmain.tile([128, 512], f32, tag="cos", name=f"cos{t}_{cc}")
                        for cc in range(NCC)]
            cps = cps_t[t]
            js = range(BPT - 1) if defer_last else range(BPT)
            for cc in range(NCC):
                for j in js:
                    for k in range(KE):
                        mm(t, cc, k, j, cps)
            if defer_last:
                for cc in range(NCC):
                    for k in range(KE):
                        mm(t, cc, k, BPT - 1, cps)
                    emit_exps(t, [cc])

        lb_sb = cpool.tile([128, T], f32, tag="lb")

        def emit_exps(t, ccs):
            """copy PSUM chunk(s) to bf16 SBUF on the (idle) DVE; frees the
            PSUM bank early and feeds ACT its fast input path. The exp for
            each half-tile [128,1024] is emitted once both its chunks are
            copied, with the class-axis sum fused via accum_out. Each half
            gets its own SBUF tile so the exp read never shares a tile with
            a concurrent DVE copy."""
            if t not in ses_t:
                ses_t[t] = cpool.tile([128, 2], f32, tag=f"ses{t}",
                                      name=f"sum2_{t}")
            cps = cps_t[t]
            for cc in ccs:
                h = cc // 2
                if (t, h) not in cosbf_t:
                    cosbf_t[(t, h)] = cpool.tile(
                        [128, 1024], bf16, tag=f"cosbf{t}_{h}",
                        name=f"cosbf{t}_{h}")
                half = cosbf_t[(t, h)]
                nc.vector.tensor_copy(half[:, 512 * (cc % 2):512 * (cc % 2 + 1)],
                                      cps[cc][:])
                if cc % 2 == 1:
                    escr = vpool.tile([128, 1024], bf16, tag="escr")
                    nc.scalar.activation(
                        escr[:], half[:],
                        AF.Exp, scale=escale[:],
                        accum_out=ses_t[t][:, h:h + 1])

        def emit_tail(t):
            """CE epilogue: one DVE add + one ACT op:
            lb = ln((s0+s1)*exp(-64ft-SHIFT) + eb2d) = ln(se2) - 64ft - SHIFT"""
            emit_eb()
            sfull = cpool.tile([128, 1], f32, tag=f"sfull{t}")
            nc.vector.tensor_tensor(sfull[:], ses_t[t][:, 0:1], ses_t[t][:, 1:2],
                                    A.add)
            nc.scalar.activation(lb_sb[:, t:t + 1], sfull[:], AF.Ln,
                                 scale=eb[:, 2 * T + t:2 * T + t + 1],
                                 bias=eb2d[:, t:t + 1])

        for t in range(T):
            emit_mains(t, defer_last=(t == T - 1))
            if t < T - 1:
                emit_exps(t, range(NCC))
            emit_tail(t)

        nc.sync.dma_start(out=out_ext[:], in_=lb_sb[:])

    nc.compile()
    return nc


def _pack(logits, labels, weight):
    """Route samples to the core owning their group; build per-core inputs."""
    logits = np.asarray(logits, dtype=np.float32)
    labels = np.asarray(labels).astype(np.int64)
    weight = np.asarray(weight, dtype=np.float32)

    group = (labels // C).astype(np.int64)
    local = (labels % C).astype(np.int64)
    core = group // GPC
    gl = group % GPC

    # host-side l2 normalization; weights are quantized to fp8 (x16 keeps
    # the values in fp8e4's normal range; cos is invariant to the row
    # scaling), x stays bf16 (PE runs mixed bf16 x fp8 at the same speed)
    xn = logits / np.maximum(
        np.sqrt(np.sum(logits * logits, axis=1, keepdims=True)), EPS)
    wn2 = np.sqrt(np.einsum("gce,gce->gc", weight, weight))[:, :, None]
    wn = weight / np.maximum(wn2, EPS)
    wq = (WS * wn).astype(FP8)                    # (G, C, E) fp8 table
    xb = xn.astype(BF16)                          # (B, E) bf16
    wtar_all = (wq[group, local].astype(np.float32) / WS).astype(BF16)

    # band assignment: per (core, local-group), ceil(count/NG) bands
    percg = [[np.nonzero((core == c) & (gl == g))[0] for g in range(GPC)]
             for c in range(NCORES)]
    nbands = [sum(max(1, -(-len(idx) // NG)) for idx in percg[c])
              for c in range(NCORES)]
    nb = max(nbands)
    nb = -(-nb // BPT) * BPT  # round up to full sample tiles
    T = nb // BPT

    in_maps = []
    valid_rows = []
    for c in range(NCORES):
        # band -> (group, sample indices)
        bands = []
        for g in range(GPC):
            idx = percg[c][g]
            nslice = max(1, -(-len(idx) // NG))
            for s in range(nslice):
                bands.append((g, idx[s * NG:(s + 1) * NG]))
        while len(bands) < nb:
            bands.append((0, np.empty(0, dtype=np.int64)))

        wt = np.empty((nb, 128, KE, C), dtype=FP8)
        xbp = np.zeros((T, 128, E), dtype=BF16)
        aux = np.zeros((128, T, NAUX), dtype=BF16)
        valid = np.zeros((128, T), dtype=bool)
        for b, (g, idx) in enumerate(bands):
            wg = wq[c * GPC + g]                     # (C, E) fp8
            for k in range(KE):
                wt[b, :, k, :] = wg[:, k * 128:(k + 1) * 128].T
            t, j = b // BPT, b % BPT
            sl = slice(NG * j, NG * j + len(idx))
            xbp[t, sl, :] = xb[idx]
            aux[sl, t, 0:E] = xb[idx]
            aux[sl, t, E:2 * E] = wtar_all[idx]
            valid[sl, t] = True
        # aux[p, t, 2E + k*128 + r] = xb[t][r, k*128+p] (transposed PE x)
        aux[:, :, 2 * E:] = np.transpose(
            xbp.reshape(T, 128, KE, 128), (3, 0, 2, 1)).reshape(128, T, KE * 128)
        in_map = {"aux": aux}
        gi = 0
        for b0 in range(0, nb - 2, 2):
            in_map[f"wt{gi}"] = np.ascontiguousarray(
                np.transpose(wt[b0:b0 + 2], (1, 0, 2, 3)))
            gi += 1
        for b in (nb - 2, nb - 1):
            for cc in range(NCC):
                in_map[f"wtp{b}_{cc}"] = np.ascontiguousarray(
                    wt[b, :, :, 512 * cc:512 * (cc + 1)])
        in_maps.append(in_map)
        valid_rows.append(valid)
    return in_maps, nb, valid_rows


def _run(logits, labels, weight, trace=False, **kw):
    from concourse.bass_utils import run_bass_kernel_spmd

    in_maps, nb, valid_rows = _pack(logits, labels, weight)
    nc = _graph_cache.get(nb)
    if nc is None:
        nc = _build(nb)
        _graph_cache[nb] = nc
    res = run_bass_kernel_spmd(nc, in_maps, core_ids=list(range(NCORES)),
                               trace=trace, **kw)
    total = sum(
        float(np.asarray(res.results[i]["out"], dtype=np.float32)[valid_rows[i]].sum())
        for i in range(NCORES)) / B + LB_SHIFT
    return np.asarray(total, dtype=np.float32), res


def kernel(logits, labels, weight):
    loss, _ = _run(logits, labels, weight)
    return loss
